# revision 17
# baseline (speedup 1.0000x reference)
"""Trainium2 Bass kernel for CustomFullyConnectedLayer (soft top-k masked linear).

out = x @ W.T where W[r, c] = A[(r-c) % n, c], A = dykstra_mask(alpha, K) * V.
(The 50-iteration Dykstra mask does NOT converge to a K-sparse mask: it has
~1288 nonzeros, so W is effectively dense and the kernel is a dense matmul.)

The mask + W construction is O(n^2) scalar work (trivial next to the
2*B*n^2 = 275 GFLOP matmul), so it runs on host in numpy; the matmul runs
on 8 NeuronCores, data-parallel over the batch dim (1024 rows per core),
in bf16 with fp32 PSUM accumulation.  PE streaming floor: 1.048M cols
@2.37GHz = ~442us/core; everything else is ramp/tail engineering.

Device kernel (per core), mapping: psum[b,r] += xT[c,b].T @ wT[c,r]
  - 8 phases, one 512-wide psum bank (r-slice) each; within a phase the
    sweep is ct-outer over all 8 batch tiles -> PE consumes input at
    ~222GB/s during the ramp (vs ~375GB/s DMA supply), so the PE never
    starves after the first chunk, and psum drains stagger naturally.
  - dummy warmup matmuls on memset scratch right after the preamble keep
    the PE busy so the HAM clock gate reaches 2.4GHz before real data
    lands, and absorb the first-matmul dispatch latency.
  - psum drains alternate Vector/Scalar so the drain rate (~2x 600ns)
    keeps ahead of the next phase's 216ns/MM sweep re-using the banks.
  - output written as bf16 (halves out traffic, host upcasts to fp32).

Negative results (measured on hw, kept for the record):
  - fp8 DoubleRow hybrid: any DoubleRow instruction in the NEFF statically
    caps the PE clock at 2.0GHz (vs 2.37GHz), wiping out the row savings
    at every error-feasible bf16/fp8 split (gate 2e-2, pure fp8 is 0.033).
  - int8/uint8 matmul: not supported by the bass stack (cost model rejects).
  - diagonal-sparsity reformulation: the mask isn't sparse (1288 diagonals),
    and even a sparse one can't beat the PE dense path on DVE/ACT.
"""

import numpy as np
import ml_dtypes

import concourse.bacc as bacc
import concourse.mybir as mybir
import concourse.tile as tile
from concourse.bass_utils import run_bass_kernel_spmd

N_CORES = 8
B_FULL = 8192
C = 4096  # in_features (contraction)
R = 4096  # out_features
BS = B_FULL // N_CORES  # 1024 per-core batch shard
TOPK_L = np.float32(0.01)
NUM_ITER = 50

P = 128
CT = C // P          # 32 contraction chunks
BT = BS // P         # 8 batch tiles per core
RBANK = 512          # psum bank width (fp32)
NPH = R // RBANK     # 8 phases (one psum-bank-wide r slice each)
N_DUMMY = 3          # warmup matmuls (N=512 each, ~0.43us cold apiece)

TRACE = False
LAST = {}

_NC_CACHE = {}


def _ensure_ntff_hook():
    """Bridge the NTFF-profile hook: this image's ``antenv`` lacks the
    ``axon_hooks`` module that ``run_bass_kernel_spmd(trace=True)`` expects,
    but the actual ctypes hook implementation ships in ``trn_agent_boot``.
    Also stub out the S3 artifact upload (no creds in-container)."""
    import sys
    import types

    try:
        import antenv

        if "antenv.axon_hooks" not in sys.modules:
            mod = types.ModuleType("antenv.axon_hooks")
            store = {"hook": None}
            mod.set_axon_ntff_profile_hook = lambda h: store.__setitem__("hook", h)
            mod.get_axon_ntff_profile_hook = lambda: store["hook"]
            sys.modules["antenv.axon_hooks"] = mod
            antenv.axon_hooks = mod
        from antenv.axon_hooks import (
            get_axon_ntff_profile_hook,
            set_axon_ntff_profile_hook,
        )

        if get_axon_ntff_profile_hook() is None:
            from trn_agent_boot.trn_boot import _ntff_profile_via_ctypes

            set_axon_ntff_profile_hook(
                _ntff_profile_via_ctypes("/opt/axon/libaxon_pjrt.so")
            )

        import concourse.bass_utils as bu

        bu.upload_artifacts = lambda tmpdir: f"file://{tmpdir}"
        return True
    except Exception as e:  # profiling is best-effort; execution must not break
        print(f"ntff hook setup failed: {e}")
        return False


def _dykstra_mask(alpha, k):
    """Numpy mirror of the reference's Dykstra soft top-k (same fp32 op order)."""
    y = (alpha / TOPK_L).astype(np.float32)
    n = y.shape[-1]
    z = y.copy()
    p = np.zeros_like(y)
    q = np.zeros_like(y)
    for _ in range(NUM_ITER):
        w = z + p
        z1 = w + (np.float32(k) - np.sum(w)) / np.float32(n)
        p = w - z1
        w2 = z1 + q
        z = np.clip(w2, np.float32(0.0), np.float32(1.0))
        q = w2 - z
    return z


def _build_wT_bf16(V, alpha_topk):
    """W[r, c] = A[(r-c) % n, c]  ->  returns W.T as contiguous bf16 [c, r]."""
    n = R
    A = (alpha_topk[:, None] * V).astype(np.float32)
    D = np.concatenate([A, A], axis=0)  # [2n, n]
    s0, s1 = D.strides
    # W[r, c] = D[n - c + r, c] : skewed strided view, no index arrays
    W_view = np.lib.stride_tricks.as_strided(
        D[n:], shape=(n, n), strides=(s0, s1 - s0)
    )
    return W_view.T.astype(ml_dtypes.bfloat16, order="C")  # [c, r]


def _chunks(sizes):
    o = 0
    for s in sizes:
        yield o, s
        o += s


def _build_nc():
    if "nc" in _NC_CACHE:
        return _NC_CACHE["nc"]

    nc = bacc.Bacc(
        "TRN2", target_bir_lowering=False, debug=False, num_devices=N_CORES
    )
    bf16 = mybir.dt.bfloat16
    f32 = mybir.dt.float32
    xT_d = nc.dram_tensor("xT", [C, BS], bf16, kind="ExternalInput")
    wT_d = nc.dram_tensor("wT", [C, R], bf16, kind="ExternalInput")
    # tile-contiguous output [phase, bt, 128, 512]: each drain DMA is one
    # linear 128KB write (vs 1KB-row scatter into [BS, R], which measured
    # ~2.5us per tile and serialized the drain chain). Host reassembles.
    out_d = nc.dram_tensor("out", [NPH, BT, P, RBANK], bf16, kind="ExternalOutput")

    xT_ap = xT_d.rearrange("(t p) b -> p t b", p=P)
    wT_ap = wT_d.rearrange("(t p) r -> p t r", p=P)

    # ct-chunk arrival schedules: fine-grained at the head so the first
    # matmul can start as early as possible; coarse after.
    FIRST_CHUNKS = [1, 1, 2, 4, 8, 8, 8]
    STEADY_CHUNKS = [8, 8, 8, 8]

    with tile.TileContext(nc) as tc:
        with (
            tc.tile_pool(name="xp", bufs=1) as xp,
            tc.tile_pool(name="wp", bufs=2) as wp,
            tc.tile_pool(name="pp", bufs=8, space="PSUM") as pp,
            tc.tile_pool(name="op", bufs=16) as op,
            tc.tile_pool(name="dp", bufs=1) as dp,
        ):
            # --- PE warmup: memset scratch, then dummy matmuls so the HAM
            # clock gate sees a busy PE from ~6.5us and un-throttles to
            # 2.4GHz before the first real matmul (~9.5us). The dummy psum
            # tile takes pool buf 0; real tiles rotate consistently after.
            dummy = dp.tile([P, P + RBANK], bf16)
            nc.gpsimd.memset(dummy[:], 0.0)
            dps = pp.tile([P, RBANK], f32, tag="ps")
            for _ in range(N_DUMMY):
                nc.tensor.matmul(
                    dps[:], dummy[:, 0:P], dummy[:, P : P + RBANK],
                    start=True, stop=True,
                )
            # pre-warm the Scalar engine's activation table (Copy) so the
            # first real drain copy doesn't pay the table load.
            dummy_o = dp.tile([P, 16], bf16, tag="do")
            nc.scalar.copy(dummy_o[:], dummy[:, 0:16])

            # --- x load, alternating granules across two queues (scalar +
            # vector): a single HWDGE queue sustains only ~213GB/s, which
            # starved the phase-0 sweeps at ct4-7. First chunk split by
            # batch so the first two matmuls start after only 64KB of x.
            # The entire phase-0 critical stream (x chunks AND W-bank-0
            # chunks, interleaved in ct order) rides the gpsimd queue (Q0):
            # the HW arbiter favors Q0 under contention (~57% share
            # measured), and a single in-order queue self-paces the ramp —
            # each sweep's 384KB arrives back-to-back at ~228GB/s vs the
            # PE's 222GB/s consumption, with no arbitration jitter.
            # Phases 1-7 W prefetch on sync (Q1), out drains on scalar (Q10).
            x_sb = xp.tile([P, CT, BS], bf16)
            w0_sb = wp.tile([P, CT, RBANK], bf16, tag="w")
            r0sl = slice(0, RBANK)
            nc.gpsimd.dma_start(out=x_sb[:, 0, 0:256], in_=xT_ap[:, 0, 0:256])
            nc.gpsimd.dma_start(out=w0_sb[:, 0:1, :], in_=wT_ap[:, 0:1, r0sl])
            nc.gpsimd.dma_start(out=x_sb[:, 0, 256:BS], in_=xT_ap[:, 0, 256:BS])
            for o, s in _chunks(FIRST_CHUNKS[1:]):
                sl = slice(1 + o, 1 + o + s)
                nc.gpsimd.dma_start(out=x_sb[:, sl, :], in_=xT_ap[:, sl, :])
                nc.gpsimd.dma_start(out=w0_sb[:, sl, :], in_=wT_ap[:, sl, r0sl])

            # --- 8 phases, one 512-wide r-slice (= one psum bank) each.
            for ph in range(NPH):
                if ph == 0:
                    w_sb = w0_sb  # already streaming on Q0, ct-interleaved
                else:
                    w_sb = wp.tile([P, CT, RBANK], bf16, tag="w")
                    rsl = slice(ph * RBANK, (ph + 1) * RBANK)
                    for o, s in _chunks(STEADY_CHUNKS):
                        nc.sync.dma_start(
                            out=w_sb[:, o : o + s, :], in_=wT_ap[:, o : o + s, rsl]
                        )
                ps_tiles = [
                    pp.tile([P, RBANK], f32, tag="ps", name=f"ps_{ph}_{bt}")
                    for bt in range(BT)
                ]
                if ph == 0:
                    # ct-outer sweep: PE consumes one (x chunk, w chunk) pair
                    # per 8 matmuls (~222GB/s demand) -> never starves on the
                    # ramp while x/W stream in. Drains alternate engines so
                    # the bank-reuse WAR for phase 1 stays ahead of its sweep.
                    for ct in range(CT):
                        for bt in range(BT):
                            nc.tensor.matmul(
                                ps_tiles[bt][:],
                                x_sb[:, ct, bt * P : (bt + 1) * P],
                                w_sb[:, ct, :],
                                start=(ct == 0),
                                stop=(ct == CT - 1),
                            )
                    for bt in range(BT):
                        ot = op.tile([P, RBANK], bf16, tag="o")
                        if bt % 2 == 0:
                            nc.vector.tensor_copy(ot[:], ps_tiles[bt][:])
                        else:
                            nc.scalar.copy(ot[:], ps_tiles[bt][:])
                        nc.scalar.dma_start(out=out_d[ph, bt], in_=ot[:])
                else:
                    # bt-major (K-contiguous): bt_k's accumulation completes
                    # k/8 through the phase, so its drain + out-DMA overlap
                    # the remaining matmuls. Only the last tile's drain
                    # trails the final matmul (vs 8 tiles = ~6us before).
                    for bt in range(BT):
                        for ct in range(CT):
                            nc.tensor.matmul(
                                ps_tiles[bt][:],
                                x_sb[:, ct, bt * P : (bt + 1) * P],
                                w_sb[:, ct, :],
                                start=(ct == 0),
                                stop=(ct == CT - 1),
                            )
                        ot = op.tile([P, RBANK], bf16, tag="o")
                        if bt % 2 == 0:
                            nc.vector.tensor_copy(ot[:], ps_tiles[bt][:])
                        else:
                            nc.scalar.copy(ot[:], ps_tiles[bt][:])
                        nc.scalar.dma_start(out=out_d[ph, bt], in_=ot[:])

    nc.compile()
    _NC_CACHE["nc"] = nc
    return nc


def kernel(x=None, V=None, alpha=None, K=None, **_unused):
    x = np.asarray(x, dtype=np.float32)
    V = np.asarray(V, dtype=np.float32)
    alpha = np.asarray(alpha, dtype=np.float32)
    k = int(np.asarray(K).item())

    mask = _dykstra_mask(alpha, k)
    wT = _build_wT_bf16(V, mask)

    x_bf = x.astype(ml_dtypes.bfloat16)
    in_maps = []
    for i in range(N_CORES):
        xs = np.ascontiguousarray(x_bf[i * BS : (i + 1) * BS].T)  # [C, BS]
        in_maps.append({"xT": xs, "wT": wT})

    nc = _build_nc()
    trace = bool(TRACE) and _ensure_ntff_hook()
    res = run_bass_kernel_spmd(
        nc, in_maps, core_ids=list(range(N_CORES)), trace=trace
    )
    LAST["exec_time_ns"] = res.exec_time_ns
    LAST["mean_exec_time_ns"] = res.mean_exec_time_ns
    LAST["trace"] = res.instructions_and_trace
    # out tiles [phase, bt, 128, 512] -> [1024, 4096] per core
    shards = [
        np.transpose(np.asarray(r["out"]), (1, 2, 0, 3)).reshape(BS, R)
        for r in res.results
    ]
    out = np.concatenate(shards, axis=0)
    return np.asarray(out, dtype=np.float32)


# revision 29
# speedup vs baseline: 1.0215x; 1.0215x over previous
"""Trainium2 Bass kernel for CustomFullyConnectedLayer (soft top-k masked linear).

out = x @ W.T where W[r, c] = A[(r-c) % n, c], A = dykstra_mask(alpha, K) * V.
(The 50-iteration Dykstra mask does NOT converge to a K-sparse mask: it has
~1288 nonzeros, so W is effectively dense and the kernel is a dense matmul.)

The mask + W construction is O(n^2) scalar work (trivial next to the
2*B*n^2 = 275 GFLOP matmul), so it runs on host in numpy; the matmul runs
on 8 NeuronCores, data-parallel over the batch dim (1024 rows per core),
in bf16 with fp32 PSUM accumulation.  PE streaming floor: 1.048M cols
@2.37GHz = ~442us/core; everything else is ramp/tail engineering.

Device kernel (per core), mapping: psum[b,r] += xT[c,b].T @ wT[c,r]
  - 8 phases, one 512-wide psum bank (r-slice) each. Phase 0 sweeps
    ct-outer over all 8 batch tiles -> PE consumes input at ~222GB/s
    during the ramp (vs ~375GB/s DMA supply), so it never starves while
    x/W stream in. Phases 1-7 run bt-major (K-contiguous per batch
    tile), so each psum tile's drain + out-DMA overlap the remaining
    matmuls and only one 128KB tile trails the final matmul.
  - 3 dummy warmup matmuls on memset scratch right after the preamble
    keep the PE busy so the HAM clock gate reaches 2.4GHz before real
    data lands, and absorb the ~1.7us first-matmul dispatch latency.
  - queue routing (measured, critical): x + out on the scalar HWDGE
    queue (time-separated), W on the sync HWDGE queue, and the gpsimd
    SWDGE queue UNUSED — when active it hogs ~300GB/s and pins each
    HWDGE queue to ~77GB/s, starving the ramp. Two HWDGE queues alone
    share ~375GB/s fairly. 1KB prewarm DMAs absorb each input queue's
    one-time ~2.5us trigger->first-byte setup.
  - W is host-retiled phase-major [8, 32, 128, 512] and out is written
    tile-contiguous [8, 8, 128, 512] (host reassembles): granule DMAs
    are linear in HBM; the [4096, 4096] layouts' 1KB-row/8KB-stride
    pattern ran ~3x slower and serialized the drain chain.
  - psum drains (fp32 psum -> bf16 SBUF -> DMA) all on the Scalar
    engine: bt-major phases leave ~7us of slack per drain, and a single
    engine avoids cross-engine copy->DMA deps.
  - output written as bf16 (halves out traffic, host upcasts to fp32).

Measured: 462.8-463.1us clean runs (PE active 95.2%, zero mid-kernel
stalls, HAM warm throughout); occasional ~468-474us when early-granule
jitter triggers a HAM re-throttle during the ramp. Baseline was 470.0.

Negative results (measured on hw, kept for the record):
  - fp8 DoubleRow hybrid: any DoubleRow instruction in the NEFF statically
    caps the PE clock at 2.0GHz (vs 2.37GHz), wiping out the row savings
    at every error-feasible bf16/fp8 split (gate 2e-2, pure fp8 is 0.033).
  - int8/uint8 matmul: not supported by the bass stack (cost model rejects).
  - diagonal-sparsity reformulation: the mask isn't sparse (1288 diagonals,
    1247 exactly 1.0 - 50 Dykstra iters don't converge), and even a sparse
    one can't beat the PE dense path using DVE/ACT elementwise ops.
  - all-ct-outer phases (v2/v3): 8 psum drains bunch at each phase
    boundary; drain-chain latency stalls the next phase's sweeps and
    HAM re-throttles at every boundary (~5.7us x 7 lost).
  - x split across two DMA queues: per-chunk arrival becomes
    max(two queues) under unfair arbitration; slower than one queue.
"""

import numpy as np
import ml_dtypes

import concourse.bacc as bacc
import concourse.mybir as mybir
import concourse.tile as tile
from concourse.bass_utils import run_bass_kernel_spmd

N_CORES = 8
B_FULL = 8192
C = 4096  # in_features (contraction)
R = 4096  # out_features
BS = B_FULL // N_CORES  # 1024 per-core batch shard
TOPK_L = np.float32(0.01)
NUM_ITER = 50

P = 128
CT = C // P          # 32 contraction chunks
BT = BS // P         # 8 batch tiles per core
RBANK = 512          # psum bank width (fp32)
NPH = R // RBANK     # 8 phases (one psum-bank-wide r slice each)
N_DUMMY = 3          # warmup matmuls (N=512 each, ~0.43us cold apiece)

TRACE = False
LAST = {}

_NC_CACHE = {}


def _ensure_ntff_hook():
    """Bridge the NTFF-profile hook: this image's ``antenv`` lacks the
    ``axon_hooks`` module that ``run_bass_kernel_spmd(trace=True)`` expects,
    but the actual ctypes hook implementation ships in ``trn_agent_boot``.
    Also stub out the S3 artifact upload (no creds in-container)."""
    import sys
    import types

    try:
        import antenv

        if "antenv.axon_hooks" not in sys.modules:
            mod = types.ModuleType("antenv.axon_hooks")
            store = {"hook": None}
            mod.set_axon_ntff_profile_hook = lambda h: store.__setitem__("hook", h)
            mod.get_axon_ntff_profile_hook = lambda: store["hook"]
            sys.modules["antenv.axon_hooks"] = mod
            antenv.axon_hooks = mod
        from antenv.axon_hooks import (
            get_axon_ntff_profile_hook,
            set_axon_ntff_profile_hook,
        )

        if get_axon_ntff_profile_hook() is None:
            from trn_agent_boot.trn_boot import _ntff_profile_via_ctypes

            set_axon_ntff_profile_hook(
                _ntff_profile_via_ctypes("/opt/axon/libaxon_pjrt.so")
            )

        import concourse.bass_utils as bu

        bu.upload_artifacts = lambda tmpdir: f"file://{tmpdir}"
        return True
    except Exception as e:  # profiling is best-effort; execution must not break
        print(f"ntff hook setup failed: {e}")
        return False


def _dykstra_mask(alpha, k):
    """Numpy mirror of the reference's Dykstra soft top-k (same fp32 op order)."""
    y = (alpha / TOPK_L).astype(np.float32)
    n = y.shape[-1]
    z = y.copy()
    p = np.zeros_like(y)
    q = np.zeros_like(y)
    for _ in range(NUM_ITER):
        w = z + p
        z1 = w + (np.float32(k) - np.sum(w)) / np.float32(n)
        p = w - z1
        w2 = z1 + q
        z = np.clip(w2, np.float32(0.0), np.float32(1.0))
        q = w2 - z
    return z


def _build_wT_bf16(V, alpha_topk):
    """W[r, c] = A[(r-c) % n, c]  ->  returns W.T as contiguous bf16 [c, r]."""
    n = R
    A = (alpha_topk[:, None] * V).astype(np.float32)
    D = np.concatenate([A, A], axis=0)  # [2n, n]
    s0, s1 = D.strides
    # W[r, c] = D[n - c + r, c] : skewed strided view, no index arrays
    W_view = np.lib.stride_tricks.as_strided(
        D[n:], shape=(n, n), strides=(s0, s1 - s0)
    )
    return W_view.T.astype(ml_dtypes.bfloat16, order="C")  # [c, r]


def _chunks(sizes):
    o = 0
    for s in sizes:
        yield o, s
        o += s


def _build_nc():
    if "nc" in _NC_CACHE:
        return _NC_CACHE["nc"]

    nc = bacc.Bacc(
        "TRN2", target_bir_lowering=False, debug=False, num_devices=N_CORES
    )
    bf16 = mybir.dt.bfloat16
    f32 = mybir.dt.float32
    xT_d = nc.dram_tensor("xT", [C, BS], bf16, kind="ExternalInput")
    # phase-major tile-contiguous W [phase, ct, 128, 512] (host pre-tiles):
    # granule reads become linear in HBM. The [C, R] layout's 1KB-row /
    # 8KB-stride pattern capped the W queue at ~77GB/s under contention
    # (vs ~300GB/s for linear streams) and starved the phase-0 ramp.
    wQ_d = nc.dram_tensor("wQ", [NPH, CT, P, RBANK], bf16, kind="ExternalInput")
    # tile-contiguous output [phase, bt, 128, 512]: each drain DMA is one
    # linear 128KB write (vs 1KB-row scatter into [BS, R], which measured
    # ~2.5us per tile and serialized the drain chain). Host reassembles.
    out_d = nc.dram_tensor("out", [NPH, BT, P, RBANK], bf16, kind="ExternalOutput")

    xT_ap = xT_d.rearrange("(t p) b -> p t b", p=P)
    wQ_ap = wQ_d.rearrange("n t p r -> n p t r")

    # ct-chunk arrival schedules: fine-grained at the head so the first
    # matmul can start as early as possible; coarse after.
    FIRST_CHUNKS = [1, 1, 2, 4, 8, 8, 8]
    X_CHUNKS = [1, 1, 2, 2, 2, 4, 4, 8, 8]  # finer: x arrival is critical
    STEADY_CHUNKS = [8, 8, 8, 8]

    with tile.TileContext(nc) as tc:
        with (
            tc.tile_pool(name="xp", bufs=1) as xp,
            tc.tile_pool(name="wp", bufs=2) as wp,
            tc.tile_pool(name="pp", bufs=8, space="PSUM") as pp,
            tc.tile_pool(name="op", bufs=16) as op,
            tc.tile_pool(name="dp", bufs=1) as dp,
        ):
            # --- queue prewarm: 1KB reads posted first on both input
            # queues absorb the one-time trigger->first-byte setup (~2.5us)
            # so the real granules stream immediately.
            pre_x = dp.tile([1, RBANK], bf16, tag="prex")
            pre_w = dp.tile([1, RBANK], bf16, tag="prew")
            nc.scalar.dma_start(out=pre_x[0:1, :], in_=xT_d[0:1, 0:RBANK])
            nc.sync.dma_start(out=pre_w[0:1, :], in_=wQ_d[0, 0, 0:1, :])

            # --- PE warmup: memset scratch, then dummy matmuls so the HAM
            # clock gate sees a busy PE from ~6.5us and un-throttles to
            # 2.4GHz before the first real matmul (~9.5us). The dummy psum
            # tile takes pool buf 0; real tiles rotate consistently after.
            dummy = dp.tile([P, P + RBANK], bf16)
            nc.gpsimd.memset(dummy[:], 0.0)
            dps = pp.tile([P, RBANK], f32, tag="ps")
            for _ in range(N_DUMMY):
                nc.tensor.matmul(
                    dps[:], dummy[:, 0:P], dummy[:, P : P + RBANK],
                    start=True, stop=True,
                )
            # pre-warm the Scalar engine's activation table (Copy) so the
            # first real drain copy doesn't pay the table load.
            dummy_o = dp.tile([P, 16], bf16, tag="do")
            nc.scalar.copy(dummy_o[:], dummy[:, 0:16])

            # x and out share the scalar HWDGE queue (Q10) — they never
            # overlap in time (x done ~45us, first drain ~65us); W rides
            # sync (Q1). The gpsimd SWDGE queue is left UNUSED: when active
            # it hogs ~300GB/s and pins each HWDGE queue to ~77GB/s; the
            # two HWDGE queues alone share ~375GB/s fairly. First x chunk
            # split by batch so the first matmuls start after only 64KB.
            x_sb = xp.tile([P, CT, BS], bf16)
            nc.scalar.dma_start(out=x_sb[:, 0, 0:256], in_=xT_ap[:, 0, 0:256])
            nc.scalar.dma_start(out=x_sb[:, 0, 256:BS], in_=xT_ap[:, 0, 256:BS])
            for o, s in _chunks(X_CHUNKS[1:]):
                sl = slice(1 + o, 1 + o + s)
                nc.scalar.dma_start(out=x_sb[:, sl, :], in_=xT_ap[:, sl, :])

            # --- 8 phases, one 512-wide r-slice (= one psum bank) each.
            for ph in range(NPH):
                w_sb = wp.tile([P, CT, RBANK], bf16, tag="w")
                for o, s in _chunks(FIRST_CHUNKS if ph == 0 else STEADY_CHUNKS):
                    nc.sync.dma_start(
                        out=w_sb[:, o : o + s, :], in_=wQ_ap[ph, :, o : o + s, :]
                    )
                ps_tiles = [
                    pp.tile([P, RBANK], f32, tag="ps", name=f"ps_{ph}_{bt}")
                    for bt in range(BT)
                ]
                if ph == 0:
                    # ct-outer sweep: PE consumes one (x chunk, w chunk) pair
                    # per 8 matmuls (~222GB/s demand) -> never starves on the
                    # ramp while x/W stream in. Drains alternate engines so
                    # the bank-reuse WAR for phase 1 stays ahead of its sweep.
                    for ct in range(CT):
                        for bt in range(BT):
                            nc.tensor.matmul(
                                ps_tiles[bt][:],
                                x_sb[:, ct, bt * P : (bt + 1) * P],
                                w_sb[:, ct, :],
                                start=(ct == 0),
                                stop=(ct == CT - 1),
                            )
                    for bt in range(BT):
                        ot = op.tile([P, RBANK], bf16, tag="o")
                        nc.scalar.copy(ot[:], ps_tiles[bt][:])
                        nc.scalar.dma_start(out=out_d[ph, bt], in_=ot[:])
                else:
                    # bt-major (K-contiguous): bt_k's accumulation completes
                    # k/8 through the phase, so its drain + out-DMA overlap
                    # the remaining matmuls. Only the last tile's drain
                    # trails the final matmul (vs 8 tiles = ~6us before).
                    for bt in range(BT):
                        for ct in range(CT):
                            nc.tensor.matmul(
                                ps_tiles[bt][:],
                                x_sb[:, ct, bt * P : (bt + 1) * P],
                                w_sb[:, ct, :],
                                start=(ct == 0),
                                stop=(ct == CT - 1),
                            )
                        ot = op.tile([P, RBANK], bf16, tag="o")
                        nc.scalar.copy(ot[:], ps_tiles[bt][:])
                        nc.scalar.dma_start(out=out_d[ph, bt], in_=ot[:])

    nc.compile()
    _NC_CACHE["nc"] = nc
    return nc


def kernel(x=None, V=None, alpha=None, K=None, **_unused):
    x = np.asarray(x, dtype=np.float32)
    V = np.asarray(V, dtype=np.float32)
    alpha = np.asarray(alpha, dtype=np.float32)
    k = int(np.asarray(K).item())

    mask = _dykstra_mask(alpha, k)
    wT = _build_wT_bf16(V, mask)
    # phase-major tile-contiguous layout [phase, ct, 128, 512]
    wQ = np.ascontiguousarray(
        wT.reshape(CT, P, NPH, RBANK).transpose(2, 0, 1, 3)
    )

    x_bf = x.astype(ml_dtypes.bfloat16)
    in_maps = []
    for i in range(N_CORES):
        xs = np.ascontiguousarray(x_bf[i * BS : (i + 1) * BS].T)  # [C, BS]
        in_maps.append({"xT": xs, "wQ": wQ})

    nc = _build_nc()
    trace = bool(TRACE) and _ensure_ntff_hook()
    res = run_bass_kernel_spmd(
        nc, in_maps, core_ids=list(range(N_CORES)), trace=trace
    )
    LAST["exec_time_ns"] = res.exec_time_ns
    LAST["mean_exec_time_ns"] = res.mean_exec_time_ns
    LAST["trace"] = res.instructions_and_trace
    # out tiles [phase, bt, 128, 512] -> [1024, 4096] per core
    shards = [
        np.transpose(np.asarray(r["out"]), (1, 2, 0, 3)).reshape(BS, R)
        for r in res.results
    ]
    out = np.concatenate(shards, axis=0)
    return np.asarray(out, dtype=np.float32)
